# Initial kernel scaffold
#
"""AASIST graph-attention + graph-pool kernel for 8 TRN2 NeuronCores.

Sharding: batch b = core//2 (4 batches), destination-node half = core%2
(rows i in [256*(core%2), 256*(core%2)+256)).  Each core computes the full
attention row block (all 512 source nodes j), h, and sigmoid scores for its
256 rows; a tiny pairwise AllGather exchanges the 256 scores inside each
batch pair so every core can compute the global top-k rank of its rows and
scatter its owned output rows (via a one-hot rank matmul).  The host sums
the two per-core partial outputs per batch (disjoint rows).

Heavy-path layout (per core):
  matmul1: z[j, o] per i via  xpadT[65,512-j-chunk].T @ (W*x_i | b) packs,
           fp16 operands, fp32 PSUM, 8 i per matmul (N=512).
  tanh:    ACT over [128, 2048] PSUM blocks (+bias folded in as K=65 row).
  v-con:   DVE fp16 mul by v, segmented 3D reduce -> logits [128 j, i] f32.
  exp/agg: ACT exp(l/2), fp32 matmul e.T??  no: agg[i,:] via lhsT=e chunks.
"""

import os
import numpy as np

N_CORES = 8
B, N, D, O = 4, 512, 64, 64
NLOC = 256  # rows per core
NKEEP = 256
TEMP = 2.0
BN_EPS = 1e-5
SELU_L = 1.0507009873554805
SELU_A = 1.6732632423543772

_CACHE = {}


def build_bass():
    import concourse.bacc as bacc
    import concourse.mybir as mybir
    import concourse.tile as tile

    f32 = mybir.dt.float32
    f16 = mybir.dt.float16
    AF = mybir.ActivationFunctionType
    ALU = mybir.AluOpType
    AX = mybir.AxisListType

    nc = bacc.Bacc("TRN2", target_bir_lowering=False, debug=False,
                   num_devices=N_CORES)

    def param(name, shape, dt=f32, out=False):
        return nc.declare_dram_parameter(name, list(shape), dt, isOutput=out)

    xtpad16 = param("xtpad16", [D + 1, N], f16)       # [x[b].T ; ones]
    xloc32 = param("xloc32", [D, NLOC])               # local rows of x, T
    wmat = param("wmat", [D, O])                      # att_proj_w
    brow16 = param("brow16", [1, 8 * O], f16)         # tile(att_proj_b, 8)
    vtile16 = param("vtile16", [1, 32 * O], f16)      # tile(att_weight, 32)
    xe = param("xe", [N, D + 1])                      # [x[b] | ones]
    wa = param("wa", [D, O])
    wn = param("wn", [D, O])
    ab = param("ab", [1, O])
    nb = param("nb", [1, O])
    gam = param("gam", [1, O])
    bet = param("bet", [1, O])
    bmu = param("bmu", [1, O])
    bvar = param("bvar", [1, O])
    pw = param("pw", [1, O])
    pb = param("pb", [1, 1])
    ident = param("ident", [128, 128])
    iota256 = param("iota256", [1, NKEEP])
    idx512 = param("idx512", [1, N])
    gidx = param("gidx", [NLOC, 1])                   # per-core global rows
    out_p = param("out", [NKEEP, O], out=True)

    NBLK = 8          # i-blocks of 32
    GI = 32           # i per block
    PACKS = NLOC // 8  # 8-i packs in wi_mega

    with tile.TileContext(nc) as tc:
        with tc.tile_pool(name="const", bufs=1) as cst, \
             tc.tile_pool(name="mega", bufs=1) as mega_p, \
             tc.tile_pool(name="abuf", bufs=3) as abuf, \
             tc.tile_pool(name="lbuf", bufs=1) as lbuf, \
             tc.tile_pool(name="post", bufs=1) as post, \
             tc.tile_pool(name="ps_main", bufs=2, space="PSUM") as psm, \
             tc.tile_pool(name="ps_small", bufs=2, space="PSUM") as pss, \
             tc.tile_pool(name="dram", bufs=1, space="DRAM") as dram:

            # ---------------- prologue: loads ----------------
            t_xtpad = cst.tile([D + 1, N], f16)
            nc.sync.dma_start(out=t_xtpad, in_=xtpad16[:, :])
            t_xloc = cst.tile([D, NLOC], f32)
            nc.sync.dma_start(out=t_xloc, in_=xloc32[:, :])
            t_w = cst.tile([D, O], f32)
            nc.sync.dma_start(out=t_w, in_=wmat[:, :])
            t_wa = cst.tile([D, O], f32)
            nc.sync.dma_start(out=t_wa, in_=wa[:, :])
            t_wn = cst.tile([D, O], f32)
            nc.sync.dma_start(out=t_wn, in_=wn[:, :])
            t_ident = cst.tile([128, 128], f32)
            nc.sync.dma_start(out=t_ident, in_=ident[:, :])
            t_xe = []
            for c in range(4):
                te = cst.tile([128, D + 1], f32, name=f"t_xe{c}")
                nc.sync.dma_start(out=te, in_=xe[128 * c:128 * (c + 1), :])
                t_xe.append(te)
            t_gidx = cst.tile([NLOC, 1], f32)
            nc.sync.dma_start(out=t_gidx, in_=gidx[:, :])

            def bcast128(dst, src_ap, w):
                # DMA-broadcast a [1, w] DRAM row across 128 partitions
                import concourse.bass as bass
                ap = bass.AP(tensor=src_ap.tensor, offset=src_ap.offset,
                             ap=[[0, 128]] + src_ap.ap[1:])
                nc.sync.dma_start(out=dst, in_=ap)

            t_vb = cst.tile([128, 32 * O], f16)
            bcast128(t_vb, vtile16[:, :], 32 * O)
            t_pwb = cst.tile([128, O], f32)
            bcast128(t_pwb, pw[:, :], O)
            t_pbb = cst.tile([128, 1], f32)
            bcast128(t_pbb, pb[:, :], 1)
            t_iota = cst.tile([128, NKEEP], f32)
            bcast128(t_iota, iota256[:, :], NKEEP)
            t_idx512 = cst.tile([128, N], f32)
            bcast128(t_idx512, idx512[:, :], N)

            # ---------------- BN affine constants ----------------
            # bnscale = gam / sqrt(var+eps); bnshift = (ab+nb-mu)*bnscale + bet
            t_bn = cst.tile([1, 6 * O], f32)  # scratch row ops
            nc.sync.dma_start(out=t_bn[:, 0:O], in_=bvar[:, :])
            nc.sync.dma_start(out=t_bn[:, O:2 * O], in_=gam[:, :])
            nc.sync.dma_start(out=t_bn[:, 2 * O:3 * O], in_=ab[:, :])
            nc.sync.dma_start(out=t_bn[:, 3 * O:4 * O], in_=nb[:, :])
            nc.sync.dma_start(out=t_bn[:, 4 * O:5 * O], in_=bmu[:, :])
            nc.sync.dma_start(out=t_bn[:, 5 * O:6 * O], in_=bet[:, :])
            t_bc = cst.tile([1, 2 * O], f32)  # [bnscale | bnshift]
            # sqrt(var+eps) via ACT Sqrt (sqrt table set loaded first)
            nc.scalar.activation(t_bc[:, 0:O], t_bn[:, 0:O], AF.Sqrt,
                                 bias=float(BN_EPS), scale=1.0)
            nc.vector.reciprocal(t_bc[:, 0:O], t_bc[:, 0:O])
            nc.vector.tensor_mul(t_bc[:, 0:O], t_bc[:, 0:O], t_bn[:, O:2 * O])
            # shift = (ab+nb-mu)*scale + bet
            nc.vector.tensor_add(t_bc[:, O:2 * O], t_bn[:, 2 * O:3 * O],
                                 t_bn[:, 3 * O:4 * O])
            nc.vector.tensor_sub(t_bc[:, O:2 * O], t_bc[:, O:2 * O],
                                 t_bn[:, 4 * O:5 * O])
            nc.vector.tensor_mul(t_bc[:, O:2 * O], t_bc[:, O:2 * O],
                                 t_bc[:, 0:O])
            nc.vector.tensor_add(t_bc[:, O:2 * O], t_bc[:, O:2 * O],
                                 t_bn[:, 5 * O:6 * O])
            # bounce to DRAM then broadcast to [128, 2*O]
            d_bn = dram.tile([1, 2 * O], f32)
            nc.sync.dma_start(out=d_bn[:, :], in_=t_bc)
            t_bnb = cst.tile([128, 2 * O], f32)
            bcast128(t_bnb, d_bn[:, :], 2 * O)

            # ---------------- wi_mega build (gpsimd) ----------------
            # wi_mega[:, 64*i : 64*(i+1)] = W * xloc[:, i] ; row 64 = b tile
            t_mega = mega_p.tile([D + 1, NLOC * O], f16)
            for p in range(PACKS):
                nc.sync.dma_start(out=t_mega[D:D + 1, 512 * p:512 * (p + 1)],
                                  in_=brow16[:, :])
            for i in range(NLOC):
                nc.gpsimd.tensor_scalar_mul(
                    t_mega[0:D, O * i:O * (i + 1)], t_w,
                    t_xloc[:, i:i + 1])

            # ---------------- main loop ----------------
            l_sb = []
            for c in range(4):
                lt = lbuf.tile([128, NLOC], f32, name=f"l_sb{c}")
                l_sb.append(lt)

            for b in range(NBLK):
                for c in range(4):
                    pg = psm.tile([128, 4 * 512], f32)
                    for s in range(4):
                        pk = 4 * b + s
                        nc.tensor.matmul(
                            pg[:, 512 * s:512 * (s + 1)],
                            t_xtpad[:, 128 * c:128 * (c + 1)],
                            t_mega[:, 512 * pk:512 * (pk + 1)],
                            start=True, stop=True)
                    t_a = abuf.tile([128, 2048], f16)
                    nc.scalar.activation(t_a, pg[:, :], AF.Tanh)
                    t_am = abuf.tile([128, 2048], f16, name="t_am")
                    nc.vector.tensor_mul(t_am, t_a, t_vb)
                    nc.vector.tensor_reduce(
                        l_sb[c][:, GI * b:GI * (b + 1)],
                        t_am.rearrange("p (i o) -> p i o", o=O),
                        AX.X, op=ALU.add)

            # ---------------- softmax + aggregation ----------------
            t_e = []
            for c in range(4):
                te = post.tile([128, NLOC], f32, name=f"t_e{c}")
                nc.scalar.activation(te, l_sb[c], AF.Exp, scale=1.0 / TEMP)
                t_e.append(te)
            agg_ps = []
            for ib in range(2):
                ap_ = pss.tile([128, D + 1], f32, name=f"agg{ib}")
                for c in range(4):
                    nc.tensor.matmul(ap_,
                                     t_e[c][:, 128 * ib:128 * (ib + 1)],
                                     t_xe[c], start=(c == 0), stop=(c == 3))
                agg_ps.append(ap_)

            h_sel = []
            t_sc = post.tile([NLOC, 1], f32)  # scores, local order
            for ib in range(2):
                # normalize agg by softmax denom (col D)
                t_rc = post.tile([128, 1], f32, name=f"rc{ib}")
                nc.vector.reciprocal(t_rc, agg_ps[ib][:, D:D + 1])
                t_an = post.tile([128, D], f32, name=f"an{ib}")
                nc.vector.tensor_scalar_mul(t_an, agg_ps[ib][:, 0:D], t_rc)
                # transpose agg_n -> [64, 128]
                p_tr = pss.tile([128, 128], f32, name=f"ptr{ib}")
                nc.tensor.transpose(p_tr, t_an, t_ident)
                t_at = post.tile([D, 128], f32, name=f"at{ib}")
                nc.vector.tensor_copy(t_at, p_tr[0:D, :])
                # h_pre = aggT.T @ wa + xlocT.T @ wn
                p_h = pss.tile([128, O], f32, name=f"ph{ib}")
                nc.tensor.matmul(p_h, t_at, t_wa, start=True, stop=False)
                nc.tensor.matmul(p_h, t_xloc[:, 128 * ib:128 * (ib + 1)],
                                 t_wn, start=False, stop=True)
                # BN affine
                t_h = post.tile([128, O], f32, name=f"th{ib}")
                nc.vector.tensor_mul(t_h, p_h, t_bnb[:, 0:O])
                nc.vector.tensor_add(t_h, t_h, t_bnb[:, O:2 * O])
                # SELU: l*relu(h) + l*a*(exp(min(h,0)) - 1)
                t_neg = post.tile([128, O], f32, name=f"tneg{ib}")
                nc.vector.tensor_scalar_min(t_neg, t_h, 0.0)
                nc.scalar.activation(t_neg, t_neg, AF.Exp)
                t_rel = post.tile([128, O], f32, name=f"trel{ib}")
                nc.vector.tensor_scalar_max(t_rel, t_h, 0.0)
                nc.vector.tensor_scalar(t_rel, t_rel, SELU_L, None,
                                        op0=ALU.mult)
                nc.vector.tensor_scalar(t_neg, t_neg, SELU_L * SELU_A,
                                        -SELU_L * SELU_A, op0=ALU.mult,
                                        op1=ALU.add)
                t_hs = post.tile([128, O], f32, name=f"ths{ib}")
                nc.vector.tensor_add(t_hs, t_rel, t_neg)
                h_sel.append(t_hs)
                # scores: sigmoid(h.pw + pb) = 1/(1+exp(-z))
                t_z = post.tile([128, O], f32, name=f"tz{ib}")
                nc.vector.tensor_mul(t_z, t_hs, t_pwb)
                t_zs = post.tile([128, 1], f32, name=f"tzs{ib}")
                nc.vector.tensor_reduce(t_zs, t_z, AX.X, op=ALU.add)
                nc.vector.tensor_add(t_zs, t_zs, t_pbb)
                nc.scalar.activation(t_zs, t_zs, AF.Exp, scale=-1.0)
                nc.vector.tensor_scalar_add(t_zs, t_zs, 1.0)
                nc.vector.reciprocal(t_sc[128 * ib:128 * (ib + 1), :], t_zs)

            # ---------------- score exchange (pairwise AllGather) --------
            d_ci = dram.tile([NLOC, 1], f32)
            nc.sync.dma_start(out=d_ci[:, :], in_=t_sc)
            d_co = dram.tile([N, 1], f32, addr_space="Shared")
            nc.gpsimd.collective_compute(
                "AllGather", nc.mybir.AluOpType.bypass,
                replica_groups=[[0, 1], [2, 3], [4, 5], [6, 7]],
                ins=[d_ci[:, :].opt()], outs=[d_co[:, :].opt()])
            t_sall = cst.tile([128, N], f32)
            import concourse.bass as bass
            sall_src = bass.AP(tensor=d_co[:, :].tensor,
                               offset=d_co[:, :].offset,
                               ap=[[0, 128], [1, N]])
            nc.sync.dma_start(out=t_sall, in_=sall_src)

            # ---------------- ranks + one-hot gather matmul ----------
            out_sb = post.tile([NKEEP, O], f32)
            pt = []
            for ib in range(2):
                t_s = t_sc[128 * ib:128 * (ib + 1), :]
                t_cmp = post.tile([128, N], f32, name=f"cmp{ib}")
                nc.vector.tensor_scalar(t_cmp, t_sall, t_s, None,
                                        op0=ALU.is_gt)
                t_rank = post.tile([128, 1], f32, name=f"rank{ib}")
                nc.vector.tensor_reduce(t_rank, t_cmp, AX.X, op=ALU.add)
                # tie-break: + #{k < gidx_p : s_all[k] == s_p}
                t_eq = post.tile([128, N], f32, name=f"eq{ib}")
                nc.vector.tensor_scalar(t_eq, t_sall, t_s, None,
                                        op0=ALU.is_equal)
                t_lt = post.tile([128, N], f32, name=f"lt{ib}")
                nc.vector.tensor_scalar(
                    t_lt, t_idx512, t_gidx[128 * ib:128 * (ib + 1), :],
                    None, op0=ALU.is_lt)
                nc.vector.tensor_mul(t_eq, t_eq, t_lt)
                t_tb = post.tile([128, 1], f32, name=f"tb{ib}")
                nc.vector.tensor_reduce(t_tb, t_eq, AX.X, op=ALU.add)
                nc.vector.tensor_add(t_rank, t_rank, t_tb)
                # PT[p, r] = (rank_p == r) * s_p
                t_pt = post.tile([128, NKEEP], f32, name=f"pt{ib}")
                nc.vector.tensor_scalar(t_pt, t_iota, t_rank, None,
                                        op0=ALU.is_equal)
                nc.vector.tensor_scalar_mul(t_pt, t_pt, t_s)
                pt.append(t_pt)
            for rb in range(2):
                p_o = pss.tile([128, O], f32, name=f"po{rb}")
                for ib in range(2):
                    nc.tensor.matmul(p_o, pt[ib][:, 128 * rb:128 * (rb + 1)],
                                     h_sel[ib], start=(ib == 0),
                                     stop=(ib == 1))
                nc.vector.tensor_copy(out_sb[128 * rb:128 * (rb + 1), :], p_o)
            nc.sync.dma_start(out=out_p[:, :], in_=out_sb)

    nc.compile()
    return nc


def make_in_maps(inputs):
    x = np.asarray(inputs["x"], dtype=np.float32)
    W = np.asarray(inputs["att_proj_w"], dtype=np.float32)
    b = np.asarray(inputs["att_proj_b"], dtype=np.float32)
    v = np.asarray(inputs["att_weight"], dtype=np.float32)[:, 0]
    in_maps = []
    for core in range(N_CORES):
        bi, half = core // 2, core % 2
        i0 = half * NLOC
        xb = x[bi]
        xtpad = np.concatenate([xb.T, np.ones((1, N), np.float32)], axis=0)
        m = {
            "xtpad16": xtpad.astype(np.float16),
            "xloc32": np.ascontiguousarray(xb[i0:i0 + NLOC].T),
            "wmat": W,
            "brow16": np.tile(b, 8)[None, :].astype(np.float16),
            "vtile16": np.tile(v, 32)[None, :].astype(np.float16),
            "xe": np.concatenate([xb, np.ones((N, 1), np.float32)], axis=1),
            "wa": np.asarray(inputs["proj_att_w"], dtype=np.float32),
            "wn": np.asarray(inputs["proj_noatt_w"], dtype=np.float32),
            "ab": np.asarray(inputs["proj_att_b"], np.float32)[None, :],
            "nb": np.asarray(inputs["proj_noatt_b"], np.float32)[None, :],
            "gam": np.asarray(inputs["bn_gamma"], np.float32)[None, :],
            "bet": np.asarray(inputs["bn_beta"], np.float32)[None, :],
            "bmu": np.asarray(inputs["bn_mean"], np.float32)[None, :],
            "bvar": np.asarray(inputs["bn_var"], np.float32)[None, :],
            "pw": np.asarray(inputs["pool_w"], np.float32)[:, 0][None, :],
            "pb": np.asarray(inputs["pool_b"], np.float32)[None, :],
            "ident": np.eye(128, dtype=np.float32),
            "iota256": np.arange(NKEEP, dtype=np.float32)[None, :],
            "idx512": np.arange(N, dtype=np.float32)[None, :],
            "gidx": (i0 + np.arange(NLOC, dtype=np.float32))[:, None],
        }
        in_maps.append(m)
    return in_maps


def run(inputs, trace=False, trace_kwargs=None):
    from concourse.bass_utils import run_bass_kernel_spmd
    if "nc" not in _CACHE:
        _CACHE["nc"] = build_bass()
    nc = _CACHE["nc"]
    in_maps = make_in_maps(inputs)
    kw = {}
    if trace:
        kw["trace"] = True
        if trace_kwargs:
            kw.update(trace_kwargs)
    res = run_bass_kernel_spmd(nc, in_maps, core_ids=list(range(N_CORES)),
                               **kw)
    outs = []
    for bi in range(B):
        o = res.results[2 * bi]["out"] + res.results[2 * bi + 1]["out"]
        outs.append(o)
    full = np.stack(outs).astype(np.float32)
    return full, res


def kernel(**inputs) -> np.ndarray:
    out, _ = run(inputs, trace=False)
    return out


# revision 19
# speedup vs baseline: 2.2991x; 2.2991x over previous
"""AASIST graph-attention + graph-pool kernel for 8 TRN2 NeuronCores.

Sharding: batch b = core//2 (4 batches), destination-row half = core%2
(rows i in [256*(core%2), 256*(core%2)+256)).  Each core computes the
attention block (its 256 rows x all 512 source nodes), h, and sigmoid
scores for its rows; a pairwise AllGather exchanges the 256 scores inside
each batch pair so every core can compute the global top-k rank of its
rows and scatter its owned output rows via a one-hot rank matmul.  The
host sums the two per-core partial outputs per batch (disjoint rows).

Heavy path per core:
  matmul1  z[j, 64*ii+o] = sum_d xpadT[d, j] * (W*x_i)[d, o] (+ bias row),
           fp16 operands, fp32 PSUM, 8 i's per matmul (N=512), PE.
  tanh     ACT over [128, 2048] PSUM blocks -> fp16 SBUF.
  v-con    DVE fp16 mul by tiled v + segmented 3D reduce -> logits f32
           in [j, i] layout (no transposes needed for aggregation).
  exp/agg  ACT exp(l/TEMP); fp32 matmuls contract j; softmax denom via
           an appended ones-column of x.
"""

import numpy as np

N_CORES = 8
B, N, D, O = 4, 512, 64, 64
NLOC = 256   # rows per core
NKEEP = 256
TEMP = 2.0
BN_EPS = 1e-5
SELU_L = 1.0507009873554805
SELU_A = 1.6732632423543772

_CACHE = {}


def build_bass():
    import concourse.bass as bass
    import concourse.bacc as bacc
    import concourse.mybir as mybir
    import concourse.tile as tile

    f32 = mybir.dt.float32
    f16 = mybir.dt.float16
    AF = mybir.ActivationFunctionType
    ALU = mybir.AluOpType
    AX = mybir.AxisListType

    nc = bacc.Bacc("TRN2", target_bir_lowering=False, debug=False,
                   num_devices=N_CORES)

    def param(name, shape, dt=f32, out=False):
        return nc.declare_dram_parameter(name, list(shape), dt, isOutput=out)

    xtpad16 = param("xtpad16", [D + 1, N], f16)       # [x[b].T ; ones]
    xloc32 = param("xloc32", [D, NLOC])               # local rows of x, T
    wmat = param("wmat", [D, O])                      # att_proj_w
    brow16 = param("brow16", [1, 8 * O], f16)         # tile(att_proj_b, 8)
    vtile16 = param("vtile16", [1, 32 * O], f16)      # tile(att_weight, 32)
    xe = param("xe", [N, D + 1])                      # [x[b] | ones]
    wa = param("wa", [D, O])
    wn = param("wn", [D, O])
    ab = param("ab", [1, O])
    nb = param("nb", [1, O])
    gam = param("gam", [1, O])
    bet = param("bet", [1, O])
    bmu = param("bmu", [1, O])
    bvar = param("bvar", [1, O])
    pw = param("pw", [1, O])
    pb = param("pb", [1, 1])
    ident = param("ident", [128, 128])
    iota256 = param("iota256", [1, NKEEP])
    idx512 = param("idx512", [1, N])
    gidx = param("gidx", [128, 2])                    # global row index cols
    out_p = param("out", [NKEEP, O], out=True)

    NBLK = 8          # i-blocks of 32
    GI = 32           # i's per block
    PACKS = NLOC // 8

    with tile.TileContext(nc) as tc:
        with tc.tile_pool(name="const", bufs=1) as cst, \
             tc.tile_pool(name="mega", bufs=1) as mega_p, \
             tc.tile_pool(name="abuf", bufs=6) as abuf, \
             tc.tile_pool(name="lbuf", bufs=1) as lbuf, \
             tc.tile_pool(name="post", bufs=1) as post, \
             tc.tile_pool(name="dram", bufs=1, space="DRAM") as dram:

            # ---------------- prologue: loads ----------------
            t_w = cst.tile([D, O], f32)
            nc.sync.dma_start(out=t_w, in_=wmat[:, :])
            t_xloc = cst.tile([D, NLOC], f32)
            nc.sync.dma_start(out=t_xloc, in_=xloc32[:, :])
            t_xtpad = cst.tile([D + 1, N], f16)
            nc.sync.dma_start(out=t_xtpad, in_=xtpad16[:, :])
            t_wa = cst.tile([D, O], f32)
            nc.sync.dma_start(out=t_wa, in_=wa[:, :])
            t_wn = cst.tile([D, O], f32)
            nc.sync.dma_start(out=t_wn, in_=wn[:, :])
            t_ident = cst.tile([128, 128], f32)
            nc.sync.dma_start(out=t_ident, in_=ident[:, :])
            t_xe = []
            for c in range(4):
                te = cst.tile([128, D + 1], f32, name=f"t_xe{c}")
                nc.sync.dma_start(out=te, in_=xe[128 * c:128 * (c + 1), :])
                t_xe.append(te)
            t_gidx = cst.tile([128, 2], f32)
            nc.sync.dma_start(out=t_gidx, in_=gidx[:, :])

            def bcast128(dst, src_ap):
                ap = bass.AP(tensor=src_ap.tensor, offset=src_ap.offset,
                             ap=[[0, 128]] + src_ap.ap[1:])
                nc.sync.dma_start(out=dst, in_=ap)

            t_vb = cst.tile([128, 32 * O], f16)
            bcast128(t_vb, vtile16[:, :])
            t_pwb = cst.tile([128, O], f32)
            bcast128(t_pwb, pw[:, :])
            t_pbb = cst.tile([128, 1], f32)
            bcast128(t_pbb, pb[:, :])
            t_iota = cst.tile([128, NKEEP], f32)
            bcast128(t_iota, iota256[:, :])
            t_idx512 = cst.tile([128, N], f32)
            bcast128(t_idx512, idx512[:, :])

            # ---------------- BN affine constants (DVE only) ----------
            # bnscale = gam * rsqrt(var+eps) via Newton from y0 = 1/(var+eps)
            # bnshift = (ab+nb-mu)*bnscale + bet
            t_bn = cst.tile([1, 6 * O], f32)
            nc.sync.dma_start(out=t_bn[:, 0:O], in_=bvar[:, :])
            nc.sync.dma_start(out=t_bn[:, O:2 * O], in_=gam[:, :])
            nc.sync.dma_start(out=t_bn[:, 2 * O:3 * O], in_=ab[:, :])
            nc.sync.dma_start(out=t_bn[:, 3 * O:4 * O], in_=nb[:, :])
            nc.sync.dma_start(out=t_bn[:, 4 * O:5 * O], in_=bmu[:, :])
            nc.sync.dma_start(out=t_bn[:, 5 * O:6 * O], in_=bet[:, :])
            t_bc = cst.tile([1, 2 * O], f32)   # [bnscale | bnshift]
            t_nt = cst.tile([1, 2 * O], f32)   # newton scratch [a | t]
            nc.vector.tensor_scalar_add(t_nt[:, 0:O], t_bn[:, 0:O],
                                        float(BN_EPS))          # a = var+eps
            nc.vector.reciprocal(t_bc[:, 0:O], t_nt[:, 0:O])    # y0 = 1/a
            for _ in range(4):
                # y <- y * (1.5 - 0.5 * a * y^2)
                nc.vector.tensor_mul(t_nt[:, O:2 * O], t_bc[:, 0:O],
                                     t_bc[:, 0:O])
                nc.vector.tensor_mul(t_nt[:, O:2 * O], t_nt[:, O:2 * O],
                                     t_nt[:, 0:O])
                nc.vector.tensor_scalar(t_nt[:, O:2 * O], t_nt[:, O:2 * O],
                                        -0.5, 1.5, op0=ALU.mult, op1=ALU.add)
                nc.vector.tensor_mul(t_bc[:, 0:O], t_bc[:, 0:O],
                                     t_nt[:, O:2 * O])
            nc.vector.tensor_mul(t_bc[:, 0:O], t_bc[:, 0:O], t_bn[:, O:2 * O])
            nc.vector.tensor_add(t_bc[:, O:2 * O], t_bn[:, 2 * O:3 * O],
                                 t_bn[:, 3 * O:4 * O])
            nc.vector.tensor_sub(t_bc[:, O:2 * O], t_bc[:, O:2 * O],
                                 t_bn[:, 4 * O:5 * O])
            nc.vector.tensor_mul(t_bc[:, O:2 * O], t_bc[:, O:2 * O],
                                 t_bc[:, 0:O])
            nc.vector.tensor_add(t_bc[:, O:2 * O], t_bc[:, O:2 * O],
                                 t_bn[:, 5 * O:6 * O])
            d_bn = dram.tile([1, 2 * O], f32)
            nc.sync.dma_start(out=d_bn[:, :], in_=t_bc)
            t_bnb = cst.tile([128, 2 * O], f32)
            bcast128(t_bnb, d_bn[:, :])

            # ---------------- wi_mega build (gpsimd, 8 chunks) --------
            t_mega = mega_p.tile([D + 1, NLOC * O], f16)
            for p in range(PACKS):
                nc.sync.dma_start(out=t_mega[D:D + 1, 512 * p:512 * (p + 1)],
                                  in_=brow16[:, :])
            CH = NLOC * O // 8   # 2048 cols = 32 i's per chunk
            CHI = NLOC // 8
            for k in range(8):
                w_b = t_w[:, :].unsqueeze(1).to_broadcast([D, CHI, O])
                x_b = (t_xloc[:, CHI * k:CHI * (k + 1)]
                       .unsqueeze(2).to_broadcast([D, CHI, O]))
                nc.gpsimd.tensor_mul(
                    t_mega[0:D, CH * k:CH * (k + 1)].rearrange(
                        "p (i o) -> p i o", o=O),
                    w_b, x_b)

            # ---------------- main loop ----------------
            l_sb = []
            for c in range(4):
                lt = lbuf.tile([128, NLOC], f16, name=f"l_sb{c}")
                l_sb.append(lt)

            with tc.tile_pool(name="ps_main", bufs=2, space="PSUM") as psm:
                for b in range(NBLK):
                    for c in range(4):
                        pg = psm.tile([128, 4 * 512], f32)
                        for s in range(4):
                            pk = 4 * b + s
                            nc.tensor.matmul(
                                pg[:, 512 * s:512 * (s + 1)],
                                t_xtpad[:, 128 * c:128 * (c + 1)],
                                t_mega[:, 512 * pk:512 * (pk + 1)],
                                start=True, stop=True)
                        t_a = abuf.tile([128, 2048], f16)
                        nc.scalar.activation(t_a, pg[:, :], AF.Tanh)
                        t_am = abuf.tile([128, 2048], f16, name="t_am")
                        nc.vector.tensor_mul(t_am, t_a, t_vb)
                        with nc.allow_low_precision(
                                reason="f16 logits, bounded |l|<1.5"):
                            nc.vector.tensor_reduce(
                                l_sb[c][:, GI * b:GI * (b + 1)],
                                t_am.rearrange("p (i o) -> p i o", o=O),
                                AX.X, op=ALU.add)

            # ---------------- softmax + aggregation ----------------
            with tc.tile_pool(name="ps_post", bufs=1, space="PSUM") as pss:
                t_e = []
                for c in range(4):
                    te = post.tile([128, NLOC], f32, name=f"t_e{c}")
                    nc.scalar.activation(te, l_sb[c], AF.Exp,
                                         scale=1.0 / TEMP)
                    t_e.append(te)
                agg_sb = []
                for ib in range(2):
                    ap_ = pss.tile([128, D + 1], f32, name=f"agg{ib}")
                    for c in range(4):
                        nc.tensor.matmul(ap_,
                                         t_e[c][:, 128 * ib:128 * (ib + 1)],
                                         t_xe[c], start=(c == 0),
                                         stop=(c == 3))
                    # normalize by softmax denom (col D)
                    t_rc = post.tile([128, 1], f32, name=f"rc{ib}")
                    nc.vector.reciprocal(t_rc, ap_[:, D:D + 1])
                    t_an = post.tile([128, D], f32, name=f"an{ib}")
                    nc.vector.tensor_scalar_mul(t_an, ap_[:, 0:D], t_rc)
                    agg_sb.append(t_an)

                h_sel = []
                t_sc = post.tile([128, 2], f32)   # scores col per i-block
                for ib in range(2):
                    p_tr = pss.tile([D, 128], f32, name=f"ptr{ib}")
                    nc.tensor.transpose(p_tr, agg_sb[ib], t_ident)
                    t_at = post.tile([D, 128], f32, name=f"at{ib}")
                    nc.vector.tensor_copy(t_at, p_tr)
                    p_h = pss.tile([128, O], f32, name=f"ph{ib}")
                    nc.tensor.matmul(p_h, t_at, t_wa, start=True, stop=False)
                    nc.tensor.matmul(p_h, t_xloc[:, 128 * ib:128 * (ib + 1)],
                                     t_wn, start=False, stop=True)
                    t_h = post.tile([128, O], f32, name=f"th{ib}")
                    nc.vector.tensor_mul(t_h, p_h, t_bnb[:, 0:O])
                    nc.vector.tensor_add(t_h, t_h, t_bnb[:, O:2 * O])
                    # SELU
                    t_neg = post.tile([128, O], f32, name=f"tneg{ib}")
                    nc.vector.tensor_scalar_min(t_neg, t_h, 0.0)
                    t_exn = post.tile([128, O], f32, name=f"texn{ib}")
                    nc.scalar.activation(t_exn, t_neg, AF.Exp)
                    t_rel = post.tile([128, O], f32, name=f"trel{ib}")
                    nc.vector.tensor_scalar_max(t_rel, t_h, 0.0)
                    nc.vector.tensor_scalar(t_rel, t_rel, SELU_L, None,
                                            op0=ALU.mult)
                    nc.vector.tensor_scalar(t_exn, t_exn, SELU_L * SELU_A,
                                            -SELU_L * SELU_A, op0=ALU.mult,
                                            op1=ALU.add)
                    t_hs = post.tile([128, O], f32, name=f"ths{ib}")
                    nc.vector.tensor_add(t_hs, t_rel, t_exn)
                    h_sel.append(t_hs)
                    # scores
                    t_z = post.tile([128, O], f32, name=f"tz{ib}")
                    nc.vector.tensor_mul(t_z, t_hs, t_pwb)
                    t_zs = post.tile([128, 1], f32, name=f"tzs{ib}")
                    nc.vector.tensor_reduce(t_zs, t_z, AX.X, op=ALU.add)
                    nc.vector.tensor_add(t_zs, t_zs, t_pbb)
                    t_ze = post.tile([128, 1], f32, name=f"tze{ib}")
                    nc.scalar.activation(t_ze, t_zs, AF.Exp, scale=-1.0)
                    nc.vector.tensor_scalar_add(t_ze, t_ze, 1.0)
                    nc.vector.reciprocal(t_sc[:, ib:ib + 1], t_ze)

                # -------- score exchange (pairwise AllGather) --------
                d_ci = dram.tile([NLOC, 1], f32)
                nc.sync.dma_start(out=d_ci[0:128, :], in_=t_sc[:, 0:1])
                nc.sync.dma_start(out=d_ci[128:256, :], in_=t_sc[:, 1:2])
                d_co = dram.tile([N, 1], f32)
                nc.gpsimd.collective_compute(
                    "AllGather", ALU.bypass,
                    replica_groups=[[0, 1], [2, 3], [4, 5], [6, 7]],
                    ins=[d_ci[:, :].opt()], outs=[d_co[:, :].opt()])
                t_sall = cst.tile([128, N], f32)
                sall_src = bass.AP(tensor=d_co[:, :].tensor,
                                   offset=d_co[:, :].offset,
                                   ap=[[0, 128], [1, N]])
                nc.sync.dma_start(out=t_sall, in_=sall_src)

                # -------- ranks + one-hot gather matmul --------
                pt = []
                for ib in range(2):
                    t_s = t_sc[:, ib:ib + 1]
                    t_cmp = post.tile([128, N], f32, name=f"cmp{ib}")
                    nc.vector.tensor_scalar(t_cmp, t_sall, t_s, None,
                                            op0=ALU.is_gt)
                    t_rank = post.tile([128, 1], f32, name=f"rank{ib}")
                    nc.vector.tensor_reduce(t_rank, t_cmp, AX.X, op=ALU.add)
                    # tie-break: + #{k < gidx_p : s_all[k] == s_p}
                    t_eq = post.tile([128, N], f32, name=f"eq{ib}")
                    nc.vector.tensor_scalar(t_eq, t_sall, t_s, None,
                                            op0=ALU.is_equal)
                    t_lt = post.tile([128, N], f32, name=f"lt{ib}")
                    nc.vector.tensor_scalar(t_lt, t_idx512,
                                            t_gidx[:, ib:ib + 1], None,
                                            op0=ALU.is_lt)
                    nc.vector.tensor_mul(t_eq, t_eq, t_lt)
                    t_tb = post.tile([128, 1], f32, name=f"tb{ib}")
                    nc.vector.tensor_reduce(t_tb, t_eq, AX.X, op=ALU.add)
                    nc.vector.tensor_add(t_rank, t_rank, t_tb)
                    # PT[p, r] = (rank_p == r) * s_p   (fused two-scalar op)
                    t_pt = post.tile([128, NKEEP], f32, name=f"pt{ib}")
                    nc.vector.tensor_scalar(t_pt, t_iota, t_rank, t_s,
                                            op0=ALU.is_equal, op1=ALU.mult)
                    pt.append(t_pt)
                for rb in range(2):
                    p_o = pss.tile([128, O], f32, name=f"po{rb}")
                    for ib in range(2):
                        nc.tensor.matmul(p_o,
                                         pt[ib][:, 128 * rb:128 * (rb + 1)],
                                         h_sel[ib], start=(ib == 0),
                                         stop=(ib == 1))
                    t_o = post.tile([128, O], f32, name=f"to{rb}")
                    nc.vector.tensor_copy(t_o, p_o)
                    nc.sync.dma_start(
                        out=out_p[128 * rb:128 * (rb + 1), :], in_=t_o)

    nc.compile()
    return nc


def make_in_maps(inputs):
    x = np.asarray(inputs["x"], dtype=np.float32)
    W = np.asarray(inputs["att_proj_w"], dtype=np.float32)
    b = np.asarray(inputs["att_proj_b"], dtype=np.float32)
    v = np.asarray(inputs["att_weight"], dtype=np.float32)[:, 0]
    in_maps = []
    for core in range(N_CORES):
        bi, half = core // 2, core % 2
        i0 = half * NLOC
        xb = x[bi]
        xtpad = np.concatenate([xb.T, np.ones((1, N), np.float32)], axis=0)
        gi = (i0 + np.arange(NLOC, dtype=np.float32)).reshape(2, 128).T
        m = {
            "xtpad16": xtpad.astype(np.float16),
            "xloc32": np.ascontiguousarray(xb[i0:i0 + NLOC].T),
            "wmat": W,
            "brow16": np.tile(b, 8)[None, :].astype(np.float16),
            "vtile16": np.tile(v, 32)[None, :].astype(np.float16),
            "xe": np.concatenate([xb, np.ones((N, 1), np.float32)], axis=1),
            "wa": np.asarray(inputs["proj_att_w"], dtype=np.float32),
            "wn": np.asarray(inputs["proj_noatt_w"], dtype=np.float32),
            "ab": np.asarray(inputs["proj_att_b"], np.float32)[None, :],
            "nb": np.asarray(inputs["proj_noatt_b"], np.float32)[None, :],
            "gam": np.asarray(inputs["bn_gamma"], np.float32)[None, :],
            "bet": np.asarray(inputs["bn_beta"], np.float32)[None, :],
            "bmu": np.asarray(inputs["bn_mean"], np.float32)[None, :],
            "bvar": np.asarray(inputs["bn_var"], np.float32)[None, :],
            "pw": np.asarray(inputs["pool_w"], np.float32)[:, 0][None, :],
            "pb": np.asarray(inputs["pool_b"], np.float32)[None, :],
            "ident": np.eye(128, dtype=np.float32),
            "iota256": np.arange(NKEEP, dtype=np.float32)[None, :],
            "idx512": np.arange(N, dtype=np.float32)[None, :],
            "gidx": np.ascontiguousarray(gi),
        }
        in_maps.append(m)
    return in_maps


def run(inputs, trace=False, trace_kwargs=None):
    from concourse.bass_utils import run_bass_kernel_spmd
    if "nc" not in _CACHE:
        _CACHE["nc"] = build_bass()
    nc = _CACHE["nc"]
    in_maps = make_in_maps(inputs)
    kw = {}
    if trace:
        kw["trace"] = True
        if trace_kwargs:
            kw.update(trace_kwargs)
    res = run_bass_kernel_spmd(nc, in_maps, core_ids=list(range(N_CORES)),
                               **kw)
    outs = []
    for bi in range(B):
        o = res.results[2 * bi]["out"] + res.results[2 * bi + 1]["out"]
        outs.append(o)
    full = np.stack(outs).astype(np.float32)
    return full, res


def kernel(**inputs) -> np.ndarray:
    out, _ = run(inputs, trace=False)
    return out
